# revision 28
# baseline (speedup 1.0000x reference)
"""Multi-head causal attention (B=4,S=2048,D=768,H=12,HD=64) on 8 Trainium2 cores.

Sharding: 4-way head tensor-parallel (3 heads/core) x 2-way batch data-parallel
(2 batches/core).  Core c: batch group bg=c//4 (batches 2bg,2bg+1), head group
hg=c%4 (heads 3hg..3hg+2).

Per-core device program (SPMD; per-core differences come only from data):
  1. q/k projections emitted transposed (qT,kT: [2 batches x 64 head-dim
     partitions, rows]) with the two batches col-tiled into one PSUM tile
     (concurrent on the PE, single PSUM->SBUF cast); v projection row-major
     with an appended ones column per head (softmax denominator rides along
     the AV matmul as psum row 64), evacuated with one strided cast.
  2. Causal attention computed transposed: S_T[k,q] = kT.T @ qT, so P=exp(S_T)
     feeds AV directly with no P transpose.  Softmax skips the running max
     (scores are O(1) at this problem's scale; exp is mathematically identical
     to the reference since softmax is shift-invariant).  The two batches of a
     head run concurrently on the PE via 64-row tile packing.  AV accumulates
     ctxU_T[65, q512] = sum_k vE.T @ P_T (row 64 = denominator l).  Normalize:
     1/l via fast-approx DVE reciprocal straight from PSUM, broadcast across
     partitions on GpSimd, one fused DVE multiply.  Causal masking is a full-
     width multiply on GpSimd (keeps DVE free); the pt pool is pre-zeroed once
     so diagonal tiles need no per-iteration memset.
  3. Per 512-row q-block (x2 batches = 1024-row chunk): 8-core AllToAll (bf16,
     128-row shards) redistributes ctx so each core holds all 768 context
     features for its own 2x128 output rows; local projection with full Wp;
     bias via a K=1 ones-outer-product matmul.

Emission is software-pipelined: qk/v projection chunks are interleaved between
attention pairs as PE filler (keeps the PE dense so HAM stays at full clock),
and out-projections are delayed to the late q-blocks where the attention inner
loop is exp-bound, so the PE never head-of-line blocks on the AllToAll round
trip.

Host side only slices/casts/transposes inputs and concatenates output shards.
"""

import sys

if "/opt/trn_rl_repo" not in sys.path:
    sys.path.insert(0, "/opt/trn_rl_repo")

import numpy as np
import ml_dtypes

BF16 = ml_dtypes.bfloat16

B, S, D = 4, 2048, 768
H, HD = 12, 64
N_CORES = 8
BL = 2          # batches per core
HL = 3          # heads per core
R = BL * S      # 4096 rows per core
KSUB = D // 128  # 6

_CACHE = {}


def _build_nc():
    import concourse.bass as bass  # noqa: F401
    import concourse.tile as tile
    from concourse import bacc, mybir

    f32 = mybir.dt.float32
    bf16 = mybir.dt.bfloat16
    EXP = mybir.ActivationFunctionType.Exp

    nc = bacc.Bacc("TRN2", target_bir_lowering=False, debug=False,
                   num_devices=N_CORES)

    xT_d = nc.dram_tensor("xT", [D, R], bf16, kind="ExternalInput").ap()
    wqk_d = nc.dram_tensor("wqk", [D, 2 * HL * HD], bf16, kind="ExternalInput").ap()
    wv_d = nc.dram_tensor("wv", [D, HL * HD], bf16, kind="ExternalInput").ap()
    wp_d = nc.dram_tensor("wp", [D, D], bf16, kind="ExternalInput").ap()
    bp_d = nc.dram_tensor("bp", [1, D], bf16, kind="ExternalInput").ap()
    mk_d = nc.dram_tensor("mk", [2, 128, 1024], bf16, kind="ExternalInput").ap()
    out_d = nc.dram_tensor("out", [4, 2, 128, D], f32, kind="ExternalOutput").ap()

    RG = [[0, 1, 2, 3, 4, 5, 6, 7]]

    with tile.TileContext(nc) as tc:
        with tc.tile_pool(name="persist", bufs=1) as per, \
             tc.tile_pool(name="dram", bufs=1, space="DRAM") as dram, \
             tc.tile_pool(name="mix_ps", bufs=2, space="PSUM") as mix_ps, \
             tc.tile_pool(name="st_ps", bufs=2, space="PSUM") as st_ps, \
             tc.tile_pool(name="av_ps", bufs=1, space="PSUM") as av_ps, \
             tc.tile_pool(name="pt", bufs=8) as ptp, \
             tc.tile_pool(name="sm", bufs=4) as sm, \
             tc.tile_pool(name="agp", bufs=4) as agp, \
             tc.tile_pool(name="outp", bufs=2) as outp:
            # ---- persistent SBUF tensors -------------------------------
            wqk = per.tile([128, KSUB, 2 * HL * HD], bf16, tag="wqk")
            wv = per.tile([128, KSUB, HL * HD], bf16, tag="wv")
            xT = per.tile([128, KSUB, R], bf16, tag="xT")
            xTr = xT_d.rearrange("(o p) r -> p o r", p=128)
            for j in range(KSUB):  # consumption order for fast PE start
                nc.sync.dma_start(
                    wqk[:, j], wqk_d.rearrange("(o p) c -> p o c", p=128)[:, j])
                nc.sync.dma_start(xT[:, j, 0:512], xTr[:, j, 0:512])
                nc.sync.dma_start(xT[:, j, S:S + 512], xTr[:, j, S:S + 512])
                nc.sync.dma_start(
                    wv[:, j], wv_d.rearrange("(o p) c -> p o c", p=128)[:, j])
            # remaining x chunks, alternating batches (b0rc, b1rc)
            for rc in range(1, 4):
                for b in range(2):
                    r0 = b * S + rc * 512
                    for j in range(KSUB):
                        nc.sync.dma_start(xT[:, j, r0:r0 + 512],
                                          xTr[:, j, r0:r0 + 512])
            masks = per.tile([128, 2, 1024], bf16, tag="mk")
            nc.sync.dma_start(masks[:], mk_d.rearrange("o p c -> p o c"))
            wp = per.tile([128, KSUB, D], bf16, tag="wp")
            nc.sync.dma_start(wp[:], wp_d.rearrange("(o p) c -> p o c", p=128))
            bp_sb = per.tile([1, D], bf16, tag="bp")
            nc.sync.dma_start(bp_sb[:], bp_d[:])
            onesP = per.tile([1, 128], bf16, tag="onesP")
            nc.vector.memset(onesP[:], 1.0)

            # pair p = head p; partitions 0:64 = batch 0, 64:128 = batch 1
            qT = per.tile([128, HL, S], bf16, tag="qT")
            kT = per.tile([128, HL, S], bf16, tag="kT")
            vE = per.tile([128, 2 * 16, HL * 65], bf16, tag="vE")
            nc.vector.memset(vE[:], 1.0)

            # pre-zero the pt pool slots once: diagonal tiles rely on the
            # full-width mask multiply to zero the not-yet-written region,
            # which requires the stale slot contents to be finite.
            pt_init = [ptp.tile([128, 2, 512], bf16, tag="pt", name=f"ptz{i}")
                       for i in range(8)]
            for t in pt_init:
                nc.gpsimd.memset(t[:], 0.0)

            warm_in = dram.tile([8 * 192, 8], bf16, tag="warm_in")
            warm_out = dram.tile([8 * 192, 8], bf16, tag="warm_out")
            a2a_in = [dram.tile([8 * 192, 128], bf16, name=f"a2ai{qb}",
                                tag=f"a2ai{qb}") for qb in range(4)]
            a2a_out = [dram.tile([8 * 192, 128], bf16, name=f"a2ao{qb}",
                                 tag=f"a2ao{qb}") for qb in range(4)]

            # ---- emission helpers --------------------------------------
            def emit_qk_ct(b, rc, ct):
                # one 128-col block of the q|k projection for batch b, row
                # chunk rc: gid 2ct+half -> qT pair gid (gid<3) else kT
                # pair gid-3.
                r0 = b * S + rc * 512
                ps = mix_ps.tile([128, 512], f32, tag="mix", name="ps")
                for j in range(KSUB):
                    nc.tensor.matmul(
                        ps[:],
                        lhsT=wqk[:, j, ct * 128:(ct + 1) * 128],
                        rhs=xT[:, j, r0:r0 + 512],
                        start=(j == 0), stop=(j == KSUB - 1))
                for half in range(2):
                    gid = 2 * ct + half
                    dest = qT if gid < 3 else kT
                    pair = gid % 3
                    nc.vector.tensor_copy(
                        dest[b * 64:(b + 1) * 64, pair,
                             rc * 512:(rc + 1) * 512],
                        ps[half * 64:(half + 1) * 64, :])

            def emit_v_tile(b, rt):
                r0 = b * S + rt * 128
                psv = mix_ps.tile([128, HL * HD], f32, tag="mix", name="psv")
                for j in range(KSUB):
                    nc.tensor.matmul(
                        psv[:], lhsT=xT[:, j, r0:r0 + 128], rhs=wv[:, j, :],
                        start=(j == 0), stop=(j == KSUB - 1))
                dst = vE[:, b * 16 + rt, :].rearrange("p (h c) -> p h c", h=HL)
                nc.vector.tensor_copy(
                    dst[:, :, 0:HD],
                    psv[:, :].rearrange("p (h c) -> p h c", c=HD))

            def emit_attn_pair(qb, pair, drain):
                n_k = 4 * (qb + 1)
                n_kp = n_k // 2
                q0 = qb * 512
                # one 2-bank tile for both batches: consecutive pairs then
                # double-buffer through the pool's 2 slots, so the next
                # pair's AV can start while this pair's normalize drains
                av2 = av_ps.tile([65, 2, 512], f32, tag="av", name="av2")
                for kp in range(n_kp):
                    drain(kp, kp == n_kp - 1)  # due-units must precede kp
                    o = kp - (n_kp - 2)  # diag pair offset; >=0 on diagonal
                    qv0 = 256 if o == 1 else 0  # valid q starts here
                    stps = [st_ps.tile([128, 2, 512], f32, tag="st",
                                       name=f"st{u}") for u in range(2)]
                    for i in range(2):
                        for u in range(2):
                            kt = 2 * kp + i
                            nc.tensor.matmul(
                                stps[u][:, i, qv0:512],
                                lhsT=kT[u * 64:(u + 1) * 64, pair,
                                        kt * 128:(kt + 1) * 128],
                                rhs=qT[u * 64:(u + 1) * 64, pair,
                                       q0 + qv0:q0 + 512],
                                start=True, stop=True)
                    for u in range(2):
                        pt = ptp.tile([128, 2, 512], bf16, tag="pt")
                        nc.scalar.activation(pt[:, :, qv0:512],
                                             stps[u][:, :, qv0:512], EXP,
                                             scale=float(HD) ** -0.5)
                        if o >= 0:
                            # full-width multiply doubles as the zero-fill
                            # of pt[:, :, 0:qv0] (mask is 0 there; slot
                            # contents are finite thanks to the pool
                            # pre-zero), replacing a per-tile memset
                            mk2 = masks[:, o, :].rearrange("p (i c) -> p i c",
                                                           i=2)
                            nc.vector.tensor_mul(pt[:], pt[:], mk2[:])
                        for i in range(2):
                            kt = 2 * kp + i
                            nc.tensor.matmul(
                                av2[:, u, :],
                                lhsT=vE[:, u * 16 + kt,
                                        pair * 65:(pair + 1) * 65],
                                rhs=pt[:, i, :],
                                start=(kp == 0 and i == 0),
                                stop=(kp == n_kp - 1 and i == 1))
                # normalize chain.  Merged across both batches (the
                # u-major av2 layout makes [.., 2, 512] views contiguous)
                # except for the very last pair, where the serial chain is
                # pure tail latency and per-u ops pipeline ~2x shorter.
                # NB: reciprocal_approx_fast is a custom DVE op and reads
                # garbage from PSUM inputs — the SBUF staging copy is load-
                # bearing, not an optimization target.
                ctxn = sm.tile([64, 2, 512], bf16, tag="ctxn", name="ctxn")
                a2v = a2a_in[qb].rearrange("(j f) c -> f j c", f=192)
                if (qb, pair) == (3, HL - 1):
                    for u in range(2):
                        lsu = sm.tile([1, 512], f32, tag="lsb", name=f"ls{u}")
                        nc.vector.tensor_copy(lsu[:], av2[64:65, u, :])
                        reu = sm.tile([1, 512], f32, tag="rec", name=f"re{u}")
                        nc.vector.reciprocal_approx_fast(reu[:], lsu[:])
                        bcu = sm.tile([64, 512], f32, tag="bcs", name=f"bc{u}")
                        nc.gpsimd.partition_broadcast(bcu[:], reu[:])
                        nc.vector.tensor_mul(ctxn[:, u, :], av2[0:64, u, :],
                                             bcu[:])
                        nc.sync.dma_start(
                            a2v[64 * pair:64 * (pair + 1), 4 * u:4 * u + 4, :],
                            ctxn[:, u, :].rearrange("p (q c) -> p q c", q=4))
                else:
                    lsb = sm.tile([1, 2, 512], f32, tag="lsb", name="lsb")
                    nc.vector.tensor_copy(lsb[:], av2[64:65, :, :])
                    rec = sm.tile([1, 2, 512], f32, tag="rec", name="rec")
                    nc.vector.reciprocal_approx_fast(rec[:], lsb[:])
                    bcs = sm.tile([64, 2, 512], f32, tag="bcs", name="bcs")
                    nc.gpsimd.partition_broadcast(bcs[:], rec[:])
                    nc.vector.tensor_mul(ctxn[:], av2[0:64, :, :], bcs[:])
                    nc.sync.dma_start(
                        a2v[64 * pair:64 * (pair + 1), :, :],
                        ctxn.rearrange("p u (q c) -> p (u q) c", q=4))
                return ctxn

            def emit_a2a(qb):
                nc.gpsimd.collective_compute(
                    "AllToAll", mybir.AluOpType.bypass,
                    ins=[a2a_in[qb][:]], outs=[a2a_out[qb][:]],
                    replica_groups=RG)

            def emit_ag_fetch(qb, ag):
                # placed where the a2a(qb) wait is already satisfied, so the
                # sync queue's FIFO head-of-line wait costs nothing
                nc.sync.dma_start(
                    ag[:], a2a_out[qb].rearrange("(o p) r -> p o r", p=128))

            def emit_outproj_blk(qb, blk, ag):
                if True:
                    osb = outp.tile([128, D], f32, tag="osb")
                    for nh in range(2):
                        po = mix_ps.tile([128, 384], f32, tag="mix", name="po")
                        n0 = nh * 384
                        for j in range(KSUB):
                            nc.tensor.matmul(po[:],
                                             lhsT=ag[:, blk * KSUB + j, :],
                                             rhs=wp[:, j, n0:n0 + 384],
                                             start=(j == 0), stop=False)
                        nc.tensor.matmul(po[:], lhsT=onesP[:],
                                         rhs=bp_sb[:, n0:n0 + 384],
                                         start=False, stop=True)
                        nc.vector.tensor_copy(osb[:, n0:n0 + 384], po[:])
                    nc.sync.dma_start(out_d[qb, blk], osb[:])

            # ---- software-pipelined emission ---------------------------
            # warmup collective: absorb ncfw first-call overhead during proj
            nc.sync.dma_start(warm_in[0:128, :], masks[:, 0, 0:8])
            nc.gpsimd.collective_compute(
                "AllToAll", mybir.AluOpType.bypass,
                ins=[warm_in[:]], outs=[warm_out[:]], replica_groups=RG)
            # prologue: everything attention qb0 needs, with keepalive
            # matmuls on the first x chunk so the PE stays above the HAM
            # activity threshold while the rest of x streams in
            def prologue_keepalive(i, n=10):
                pks = mix_ps.tile([128, 512], f32, tag="mix", name=f"pk{i}")
                for w in range(n):
                    nc.tensor.matmul(pks[:], lhsT=wqk[:, 0, 0:128],
                                     rhs=xT[:, 0, 0:512],
                                     start=(w == 0), stop=(w == n - 1))
                pkb = sm.tile([128, 8], bf16, tag="ksb", name=f"pkb{i}")
                nc.vector.tensor_copy(pkb[:], pks[:, 0:8])
                nc.sync.dma_start(warm_in[0:128, :], pkb[:])

            for b in range(2):
                for ct in range(3):
                    emit_qk_ct(b, 0, ct)
                prologue_keepalive(b * 3 + 0)
            for rt in range(4):
                emit_v_tile(0, rt)
                emit_v_tile(1, rt)
                prologue_keepalive(10 + rt)

            # filler queue: (deadline=(qb,pair,kp), emit_fn), kept in
            # deadline order; before each k-pair all units due by then are
            # drained (hard ordering requirement: a unit must be emitted
            # before the attention that consumes its output), plus one unit
            # opportunistically per k-pair to spread PE filler.
            from collections import deque
            fq = deque()
            for rc in range(1, 4):
                for b in range(2):
                    for ct in range(3):
                        # qT/kT rows rc needed from (qb=rc, pair0, kp0)
                        fq.append(((rc, 0, 0), lambda b=b, rc=rc, ct=ct:
                                   emit_qk_ct(b, rc, ct)))
                for rt in range(4 * rc, 4 * rc + 4):
                    for b in range(2):
                        # vE row-tile rt consumed at kp=rt//2 of (qb=rc,pair0)
                        dl = (rc, 0, max(0, rt // 2 - 1))
                        fq.append((dl, lambda b=b, rt=rt: emit_v_tile(b, rt)))
            fq = deque(sorted(fq, key=lambda t: t[0]))

            def drain(n, due=None):
                k = 0
                while fq and (k < n or (due and fq[0][0] <= due)):
                    fq.popleft()[1]()
                    k += 1

            last_ctxn = None
            for qb in range(4):
                for pair in range(HL):
                    # extra opportunistic unit on the last kp covers the
                    # pair-boundary normalize tail with PE work
                    last_ctxn = emit_attn_pair(
                        qb, pair, lambda kp, last, qb=qb, pair=pair:
                        drain(2 if last else 1, due=(qb, pair, kp)))
                    if qb >= 2 and not (qb == 3 and pair == HL - 1) \
                            and len(fq) < 2:
                        # filler is exhausted by qb3; keep the PE busy with
                        # HAM-keepalive matmuls so the exp-bound inner loop
                        # does not let the clock gate drop to 4/8 (which
                        # would double every subsequent matmul).  Reading
                        # this pair's ctxn pins them to the right window.
                        kps = mix_ps.tile([128, 512], f32, tag="mix",
                                          name=f"kps{qb}_{pair}")
                        for w in range(24):
                            nc.tensor.matmul(
                                kps[:], lhsT=kT[0:64, 0, 0:128],
                                rhs=last_ctxn[:, 0, :],
                                start=(w == 0), stop=(w == 23))
                        ksb = sm.tile([128, 8], bf16, tag="ksb",
                                      name=f"ksb{qb}_{pair}")
                        nc.vector.tensor_copy(ksb[:], kps[:, 0:8])
                        nc.sync.dma_start(warm_in[0:128, :], ksb[:])
                if qb >= 1:
                    # fetch ag of the previous q-block now: its a2a finished
                    # during this q-block's attention, so the sync-queue wait
                    # is free; the MM bodies become late-phase PE filler
                    # (out-proj of qb 0 fills qb2, of qb 1 fills qb3, of qb 2
                    # shadows the final AllToAll)
                    qp = qb - 1
                    ag = agp.tile([128, 2 * KSUB, 128], bf16, tag="ag",
                                  name=f"ag{qp}")
                    emit_ag_fetch(qp, ag)
                    if qp <= 1:
                        dqb = qp + 2
                        fq.append(((dqb, 1, 98), lambda q=qp, a=ag:
                                   emit_outproj_blk(q, 0, a)))
                        fq.append(((dqb, 2, 98), lambda q=qp, a=ag:
                                   emit_outproj_blk(q, 1, a)))
                    else:
                        drain(99)
                        emit_outproj_blk(qp, 0, ag)
                        emit_outproj_blk(qp, 1, ag)
                emit_a2a(qb)
            # HAM-warming matmuls pinned to the final-AllToAll window: they
            # read the last pair's ctxn, so the greedy scheduler cannot hoist
            # them earlier; they keep the PE at full clock through the a2a so
            # the last out-projection runs warm.
            wsb = per.tile([128, 8], bf16, tag="wsb")
            wps = mix_ps.tile([128, 512], f32, tag="mix", name="wps")
            for w in range(48):
                nc.tensor.matmul(wps[:],
                                 lhsT=kT[0:64, 0, 0:128],
                                 rhs=last_ctxn[:, 0, :],
                                 start=(w == 0), stop=(w == 47))
            nc.vector.tensor_copy(wsb[:], wps[:, 0:8])
            nc.sync.dma_start(warm_in[0:128, :], wsb[:])
            ag3 = agp.tile([128, 2 * KSUB, 128], bf16, tag="ag", name="ag3")
            emit_ag_fetch(3, ag3)
            emit_outproj_blk(3, 0, ag3)
            emit_outproj_blk(3, 1, ag3)

    nc.compile()
    return nc


def _get_nc():
    if "nc" not in _CACHE:
        _CACHE["nc"] = _build_nc()
    return _CACHE["nc"]


def _masks_np():
    k = np.arange(128)[:, None]
    q = np.arange(512)[None, :]
    tiles = [(q >= k + 128 * t) for t in range(4)]
    m = np.stack([np.concatenate([tiles[2 * o], tiles[2 * o + 1]], axis=1)
                  for o in range(2)])
    return m.astype(BF16)


def _prep_in_maps(x, Wq, Wk, Wv, Wp, bp):
    x = np.asarray(x, dtype=np.float32)
    mk = _masks_np()
    wp_full = np.asarray(Wp).astype(BF16)
    bp_row = np.asarray(bp, dtype=np.float32).reshape(1, D).astype(BF16)
    xT_bg = []
    for bg in range(2):
        xl = x[2 * bg:2 * bg + 2].reshape(R, D)
        xT_bg.append(np.ascontiguousarray(xl.T).astype(BF16))
    wqk_hg, wv_hg = [], []
    for hg in range(4):
        hs = slice(192 * hg, 192 * (hg + 1))
        wqk_hg.append(np.concatenate(
            [np.asarray(Wq)[:, hs], np.asarray(Wk)[:, hs]], axis=1).astype(BF16))
        wv_hg.append(np.asarray(Wv)[:, hs].astype(BF16))
    in_maps = []
    for c in range(N_CORES):
        bg, hg = c // 4, c % 4
        in_maps.append({
            "xT": xT_bg[bg],
            "wqk": wqk_hg[hg],
            "wv": wv_hg[hg],
            "wp": wp_full,
            "bp": bp_row,
            "mk": mk,
        })
    return in_maps


def kernel(x, Wq, Wk, Wv, Wp, bp):
    from concourse import bass_utils

    nc = _get_nc()
    in_maps = _prep_in_maps(x, Wq, Wk, Wv, Wp, bp)
    res = bass_utils.run_bass_kernel_spmd(nc, in_maps,
                                          core_ids=list(range(N_CORES)))
    out = np.empty((B, S, D), np.float32)
    for c in range(N_CORES):
        sh = res.results[c]["out"]  # [4 chunks, 2 blocks, 128, D]
        for qb in range(4):
            for blk in range(2):
                batch = 2 * blk + c // 4
                s0 = 512 * qb + 128 * (c % 4)
                out[batch, s0:s0 + 128] = sh[qb, blk]
    return out


# revision 29
# speedup vs baseline: 1.2377x; 1.2377x over previous
"""Multi-head causal attention (B=4,S=2048,D=768,H=12,HD=64) on 8 Trainium2 cores.

Sharding: 4-way head tensor-parallel (3 heads/core) x 2-way batch data-parallel
(2 batches/core).  Core c: batch group bg=c//4 (batches 2bg,2bg+1), head group
hg=c%4 (heads 3hg..3hg+2).

Per-core device program (SPMD; per-core differences come only from data):
  1. q/k projections emitted transposed (qT,kT: [64 head-dim partitions, rows]);
     v projection row-major with an appended ones column per head (softmax
     denominator rides along the AV matmul as psum row 64).
  2. Causal attention computed transposed: S_T[k,q] = kT.T @ qT, so P=exp(S_T)
     feeds AV directly with no P transpose.  Softmax skips the running max
     (scores are O(1) at this problem's scale; exp is mathematically identical
     to the reference since softmax is shift-invariant).  The two batches of a
     head run concurrently on the PE via 64-row tile packing.  AV accumulates
     ctxU_T[65, q512] = sum_k vE.T @ P_T (row 64 = denominator l).  Normalize:
     1/l via fast-approx DVE reciprocal, broadcast across partitions on GpSimd,
     one fused DVE multiply.
  3. Per 512-row q-block (x2 batches = 1024-row chunk): 8-core AllToAll (bf16,
     128-row shards) redistributes ctx so each core holds all 768 context
     features for its own 2x128 output rows; local projection with full Wp;
     bias via a K=1 ones-outer-product matmul.

Emission is software-pipelined: qk/v projection chunks are interleaved between
attention pairs as PE filler (keeps the PE dense so HAM stays at full clock),
and each chunk's out-projection is emitted one q-block later so the PE never
head-of-line blocks on the AllToAll round trip.

Host side only slices/casts/transposes inputs and concatenates output shards.
"""

import sys

if "/opt/trn_rl_repo" not in sys.path:
    sys.path.insert(0, "/opt/trn_rl_repo")

import numpy as np
import ml_dtypes

BF16 = ml_dtypes.bfloat16

B, S, D = 4, 2048, 768
H, HD = 12, 64
N_CORES = 8
BL = 2          # batches per core
HL = 3          # heads per core
R = BL * S      # 4096 rows per core
KSUB = D // 128  # 6

_CACHE = {}


def _build_nc():
    import concourse.bass as bass  # noqa: F401
    import concourse.tile as tile
    from concourse import bacc, mybir

    f32 = mybir.dt.float32
    bf16 = mybir.dt.bfloat16
    EXP = mybir.ActivationFunctionType.Exp

    nc = bacc.Bacc("TRN2", target_bir_lowering=False, debug=False,
                   num_devices=N_CORES)

    xT_d = nc.dram_tensor("xT", [D, R], bf16, kind="ExternalInput").ap()
    wqk_d = nc.dram_tensor("wqk", [D, 2 * HL * HD], bf16, kind="ExternalInput").ap()
    wv_d = nc.dram_tensor("wv", [D, HL * HD], bf16, kind="ExternalInput").ap()
    wp_d = nc.dram_tensor("wp", [D, D], bf16, kind="ExternalInput").ap()
    bp_d = nc.dram_tensor("bp", [1, D], bf16, kind="ExternalInput").ap()
    mk_d = nc.dram_tensor("mk", [2, 128, 1024], bf16, kind="ExternalInput").ap()
    out_d = nc.dram_tensor("out", [4, 2, 128, D], f32, kind="ExternalOutput").ap()

    RG = [[0, 1, 2, 3, 4, 5, 6, 7]]

    with tile.TileContext(nc) as tc:
        with tc.tile_pool(name="persist", bufs=1) as per, \
             tc.tile_pool(name="dram", bufs=1, space="DRAM") as dram, \
             tc.tile_pool(name="mix_ps", bufs=2, space="PSUM") as mix_ps, \
             tc.tile_pool(name="st_ps", bufs=2, space="PSUM") as st_ps, \
             tc.tile_pool(name="av_ps", bufs=2, space="PSUM") as av_ps, \
             tc.tile_pool(name="pt", bufs=8) as ptp, \
             tc.tile_pool(name="sm", bufs=4) as sm, \
             tc.tile_pool(name="agp", bufs=2) as agp, \
             tc.tile_pool(name="outp", bufs=2) as outp:
            # ---- persistent SBUF tensors -------------------------------
            wqk = per.tile([128, KSUB, 2 * HL * HD], bf16, tag="wqk")
            wv = per.tile([128, KSUB, HL * HD], bf16, tag="wv")
            xT = per.tile([128, KSUB, R], bf16, tag="xT")
            xTr = xT_d.rearrange("(o p) r -> p o r", p=128)
            for j in range(KSUB):  # consumption order for fast PE start
                nc.sync.dma_start(
                    wqk[:, j], wqk_d.rearrange("(o p) c -> p o c", p=128)[:, j])
                nc.sync.dma_start(xT[:, j, 0:512], xTr[:, j, 0:512])
                nc.sync.dma_start(
                    wv[:, j], wv_d.rearrange("(o p) c -> p o c", p=128)[:, j])
            for rc in range(1, 8):
                for j in range(KSUB):
                    nc.sync.dma_start(xT[:, j, rc * 512:rc * 512 + 512],
                                      xTr[:, j, rc * 512:rc * 512 + 512])
            masks = per.tile([128, 2, 1024], bf16, tag="mk")
            nc.sync.dma_start(masks[:], mk_d.rearrange("o p c -> p o c"))
            wp = per.tile([128, KSUB, D], bf16, tag="wp")
            nc.sync.dma_start(wp[:], wp_d.rearrange("(o p) c -> p o c", p=128))
            bp_sb = per.tile([1, D], bf16, tag="bp")
            nc.sync.dma_start(bp_sb[:], bp_d[:])
            onesP = per.tile([1, 128], bf16, tag="onesP")
            nc.vector.memset(onesP[:], 1.0)

            # pair p = head p; partitions 0:64 = batch 0, 64:128 = batch 1
            qT = per.tile([128, HL, S], bf16, tag="qT")
            kT = per.tile([128, HL, S], bf16, tag="kT")
            vE = per.tile([128, 2 * 16, HL * 65], bf16, tag="vE")
            nc.vector.memset(vE[:], 1.0)

            warm_in = dram.tile([8 * 192, 8], bf16, tag="warm_in")
            warm_out = dram.tile([8 * 192, 8], bf16, tag="warm_out")
            a2a_in = [dram.tile([8 * 192, 128], bf16, name=f"a2ai{qb}",
                                tag=f"a2ai{qb}") for qb in range(4)]
            a2a_out = [dram.tile([8 * 192, 128], bf16, name=f"a2ao{qb}",
                                 tag=f"a2ao{qb}") for qb in range(4)]

            # ---- emission helpers --------------------------------------
            def emit_qk_ct(b, rc, ct):
                r0 = b * S + rc * 512
                if True:
                    ps = mix_ps.tile([128, 512], f32, tag="mix", name="ps")
                    for j in range(KSUB):
                        nc.tensor.matmul(
                            ps[:],
                            lhsT=wqk[:, j, ct * 128:(ct + 1) * 128],
                            rhs=xT[:, j, r0:r0 + 512],
                            start=(j == 0), stop=(j == KSUB - 1))
                    for half in range(2):
                        gid = 2 * ct + half
                        dest = qT if gid < 3 else kT
                        pair = gid % 3
                        nc.vector.tensor_copy(
                            dest[b * 64:(b + 1) * 64, pair,
                                 rc * 512:(rc + 1) * 512],
                            ps[half * 64:(half + 1) * 64, :])

            def emit_qk_chunk(b, rc):
                for ct in range(3):
                    emit_qk_ct(b, rc, ct)

            def emit_v_tile(b, rt):
                r0 = b * S + rt * 128
                psv = mix_ps.tile([128, HL * HD], f32, tag="mix", name="psv")
                for j in range(KSUB):
                    nc.tensor.matmul(
                        psv[:], lhsT=xT[:, j, r0:r0 + 128], rhs=wv[:, j, :],
                        start=(j == 0), stop=(j == KSUB - 1))
                for h in range(HL):
                    nc.vector.tensor_copy(
                        vE[:, b * 16 + rt, h * 65:h * 65 + 64],
                        psv[:, h * 64:(h + 1) * 64])

            def emit_attn_pair(qb, pair, drain):
                n_k = 4 * (qb + 1)
                n_kp = n_k // 2
                q0 = qb * 512
                avs = [av_ps.tile([65, 512], f32, tag="av", name=f"av{u}")
                       for u in range(2)]
                for kp in range(n_kp):
                    drain(kp)  # due-units for this kp must precede it
                    o = kp - (n_kp - 2)  # diag pair offset; >=0 on diagonal
                    qv0 = 256 if o == 1 else 0  # valid q starts here
                    stps = [st_ps.tile([128, 2, 512], f32, tag="st",
                                       name=f"st{u}") for u in range(2)]
                    for i in range(2):
                        for u in range(2):
                            kt = 2 * kp + i
                            nc.tensor.matmul(
                                stps[u][:, i, qv0:512],
                                lhsT=kT[u * 64:(u + 1) * 64, pair,
                                        kt * 128:(kt + 1) * 128],
                                rhs=qT[u * 64:(u + 1) * 64, pair,
                                       q0 + qv0:q0 + 512],
                                start=True, stop=True)
                    for u in range(2):
                        pt = ptp.tile([128, 2, 512], bf16, tag="pt")
                        if qv0:
                            nc.vector.memset(pt[:, :, 0:qv0], 0.0)
                        nc.scalar.activation(pt[:, :, qv0:512],
                                             stps[u][:, :, qv0:512], EXP,
                                             scale=float(HD) ** -0.5)
                        if o >= 0:
                            mk2 = masks[:, o, :].rearrange("p (i c) -> p i c",
                                                           i=2)
                            nc.vector.tensor_mul(pt[:, :, qv0:512],
                                                 pt[:, :, qv0:512],
                                                 mk2[:, :, qv0:512])
                        for i in range(2):
                            kt = 2 * kp + i
                            nc.tensor.matmul(
                                avs[u][:],
                                lhsT=vE[:, u * 16 + kt,
                                        pair * 65:(pair + 1) * 65],
                                rhs=pt[:, i, :],
                                start=(kp == 0 and i == 0),
                                stop=(kp == n_kp - 1 and i == 1))
                lsbs, recs, bcss, ctxns = [], [], [], []
                for u in range(2):
                    lsbs.append(sm.tile([1, 512], f32, tag="lsb",
                                        name=f"lsb{u}"))
                    nc.vector.tensor_copy(lsbs[u][:], avs[u][64:65, :])
                for u in range(2):
                    recs.append(sm.tile([1, 512], f32, tag="rec",
                                        name=f"rec{u}"))
                    nc.vector.reciprocal_approx_fast(recs[u][:], lsbs[u][:])
                for u in range(2):
                    bcss.append(sm.tile([64, 512], f32, tag="bcs",
                                        name=f"bcs{u}"))
                    nc.gpsimd.partition_broadcast(bcss[u][:], recs[u][:])
                for u in range(2):
                    ctxn = sm.tile([64, 512], bf16, tag="ctxn",
                                   name=f"ctxn{u}")
                    ctxns.append(ctxn)
                    nc.vector.tensor_mul(ctxn[:], avs[u][0:64, :], bcss[u][:])
                    a2v = a2a_in[qb].rearrange("(j f) c -> f j c", f=192)
                    nc.sync.dma_start(
                        a2v[64 * pair:64 * (pair + 1), 4 * u:4 * u + 4, :],
                        ctxns[u].rearrange("p (q c) -> p q c", q=4))

            def emit_a2a(qb):
                nc.gpsimd.collective_compute(
                    "AllToAll", mybir.AluOpType.bypass,
                    ins=[a2a_in[qb][:]], outs=[a2a_out[qb][:]],
                    replica_groups=RG)

            def emit_outproj_blk(qb, blk, ag):
                if blk == 0:
                    nc.sync.dma_start(
                        ag[:], a2a_out[qb].rearrange("(o p) r -> p o r", p=128))
                if True:
                    osb = outp.tile([128, D], f32, tag="osb")
                    for nh in range(2):
                        po = mix_ps.tile([128, 384], f32, tag="mix", name="po")
                        n0 = nh * 384
                        for j in range(KSUB):
                            nc.tensor.matmul(po[:],
                                             lhsT=ag[:, blk * KSUB + j, :],
                                             rhs=wp[:, j, n0:n0 + 384],
                                             start=(j == 0), stop=False)
                        nc.tensor.matmul(po[:], lhsT=onesP[:],
                                         rhs=bp_sb[:, n0:n0 + 384],
                                         start=False, stop=True)
                        nc.vector.tensor_copy(osb[:, n0:n0 + 384], po[:])
                    nc.sync.dma_start(out_d[qb, blk], osb[:])

            # ---- software-pipelined emission ---------------------------
            # warmup collective: absorb ncfw first-call overhead during proj
            nc.sync.dma_start(warm_in[0:128, :], masks[:, 0, 0:8])
            nc.gpsimd.collective_compute(
                "AllToAll", mybir.AluOpType.bypass,
                ins=[warm_in[:]], outs=[warm_out[:]], replica_groups=RG)
            # prologue: everything attention qb0 needs
            emit_qk_chunk(0, 0)
            emit_qk_chunk(1, 0)
            for rt in range(4):
                emit_v_tile(0, rt)
                emit_v_tile(1, rt)

            # filler queue: (deadline=(qb,pair,kp), emit_fn), kept in
            # deadline order; before each k-pair all units due by then are
            # drained (hard ordering requirement: a unit must be emitted
            # before the attention that consumes its output), plus one unit
            # opportunistically per k-pair to spread PE filler.
            from collections import deque
            fq = deque()
            for rc in range(1, 4):
                for b in range(2):
                    for ct in range(3):
                        # qT/kT rows rc needed from (qb=rc, pair0, kp0)
                        fq.append(((rc, 0, 0), lambda b=b, rc=rc, ct=ct:
                                   emit_qk_ct(b, rc, ct)))
                for rt in range(4 * rc, 4 * rc + 4):
                    for b in range(2):
                        # vE row-tile rt consumed at kp=rt//2 of (qb=rc,pair0)
                        dl = (rc, 0, max(0, rt // 2 - 1))
                        fq.append((dl, lambda b=b, rt=rt: emit_v_tile(b, rt)))
            fq = deque(sorted(fq, key=lambda t: t[0]))

            def drain(n, due=None):
                k = 0
                while fq and (k < n or (due and fq[0][0] <= due)):
                    fq.popleft()[1]()
                    k += 1

            for qb in range(4):
                for pair in range(HL):
                    emit_attn_pair(qb, pair, lambda kp, qb=qb, pair=pair:
                                   drain(1, due=(qb, pair, kp)))
                    # out-proj of chunk q consumed ~2 chunks later so the
                    # big late q-blocks keep PE filler (A2A long done)
                    op_sched = {(2, 0): 0, (3, 0): 1, (3, 1): 2}
                    if (qb, pair) in op_sched:
                        q = op_sched[(qb, pair)]
                        ag = agp.tile([128, 2 * KSUB, 128], bf16, tag="ag",
                                      name=f"ag{q}")
                        fq.append(((qb, pair, 98), lambda q=q, a=ag:
                                   emit_outproj_blk(q, 0, a)))
                        fq.append(((qb, pair, 99), lambda q=q, a=ag:
                                   emit_outproj_blk(q, 1, a)))
                emit_a2a(qb)
            drain(99)
            ag3 = agp.tile([128, 2 * KSUB, 128], bf16, tag="ag", name="ag3")
            emit_outproj_blk(3, 0, ag3)
            emit_outproj_blk(3, 1, ag3)

    nc.compile()
    return nc


def _get_nc():
    if "nc" not in _CACHE:
        _CACHE["nc"] = _build_nc()
    return _CACHE["nc"]


def _masks_np():
    k = np.arange(128)[:, None]
    q = np.arange(512)[None, :]
    tiles = [(q >= k + 128 * t) for t in range(4)]
    m = np.stack([np.concatenate([tiles[2 * o], tiles[2 * o + 1]], axis=1)
                  for o in range(2)])
    return m.astype(BF16)


def _prep_in_maps(x, Wq, Wk, Wv, Wp, bp):
    x = np.asarray(x, dtype=np.float32)
    mk = _masks_np()
    wp_full = np.asarray(Wp).astype(BF16)
    bp_row = np.asarray(bp, dtype=np.float32).reshape(1, D).astype(BF16)
    xT_bg = []
    for bg in range(2):
        xl = x[2 * bg:2 * bg + 2].reshape(R, D)
        xT_bg.append(np.ascontiguousarray(xl.T).astype(BF16))
    wqk_hg, wv_hg = [], []
    for hg in range(4):
        hs = slice(192 * hg, 192 * (hg + 1))
        wqk_hg.append(np.concatenate(
            [np.asarray(Wq)[:, hs], np.asarray(Wk)[:, hs]], axis=1).astype(BF16))
        wv_hg.append(np.asarray(Wv)[:, hs].astype(BF16))
    in_maps = []
    for c in range(N_CORES):
        bg, hg = c // 4, c % 4
        in_maps.append({
            "xT": xT_bg[bg],
            "wqk": wqk_hg[hg],
            "wv": wv_hg[hg],
            "wp": wp_full,
            "bp": bp_row,
            "mk": mk,
        })
    return in_maps


def kernel(x, Wq, Wk, Wv, Wp, bp):
    from concourse import bass_utils

    nc = _get_nc()
    in_maps = _prep_in_maps(x, Wq, Wk, Wv, Wp, bp)
    res = bass_utils.run_bass_kernel_spmd(nc, in_maps,
                                          core_ids=list(range(N_CORES)))
    out = np.empty((B, S, D), np.float32)
    for c in range(N_CORES):
        sh = res.results[c]["out"]  # [4 chunks, 2 blocks, 128, D]
        for qb in range(4):
            for blk in range(2):
                batch = 2 * blk + c // 4
                s0 = 512 * qb + 128 * (c % 4)
                out[batch, s0:s0 + 128] = sh[qb, blk]
    return out



# revision 30
# speedup vs baseline: 1.3072x; 1.0562x over previous
"""Multi-head causal attention (B=4,S=2048,D=768,H=12,HD=64) on 8 Trainium2 cores.

Sharding: 4-way head tensor-parallel (3 heads/core) x 2-way batch data-parallel
(2 batches/core).  Core c: batch group bg=c//4 (batches 2bg,2bg+1), head group
hg=c%4 (heads 3hg..3hg+2).

Per-core device program (SPMD; per-core differences come only from data):
  1. q/k projections emitted transposed (qT,kT: [64 head-dim partitions, rows]);
     v projection row-major with an appended ones column per head (softmax
     denominator rides along the AV matmul as psum row 64).
  2. Causal attention computed transposed: S_T[k,q] = kT.T @ qT, so P=exp(S_T)
     feeds AV directly with no P transpose.  Softmax skips the running max
     (scores are O(1) at this problem's scale; exp is mathematically identical
     to the reference since softmax is shift-invariant).  The two batches of a
     head run concurrently on the PE via 64-row tile packing.  AV accumulates
     ctxU_T[65, q512] = sum_k vE.T @ P_T (row 64 = denominator l).  Normalize:
     1/l via fast-approx DVE reciprocal, broadcast across partitions on GpSimd,
     one fused DVE multiply.
  3. Per 512-row q-block (x2 batches = 1024-row chunk): 8-core AllToAll (bf16,
     128-row shards) redistributes ctx so each core holds all 768 context
     features for its own 2x128 output rows; local projection with full Wp;
     bias via a K=1 ones-outer-product matmul.

Emission is software-pipelined: qk/v projection chunks are interleaved between
attention pairs as PE filler (keeps the PE dense so HAM stays at full clock),
and each chunk's out-projection is emitted one q-block later so the PE never
head-of-line blocks on the AllToAll round trip.

Host side only slices/casts/transposes inputs and concatenates output shards.
"""

import sys

if "/opt/trn_rl_repo" not in sys.path:
    sys.path.insert(0, "/opt/trn_rl_repo")

import numpy as np
import ml_dtypes

BF16 = ml_dtypes.bfloat16

B, S, D = 4, 2048, 768
H, HD = 12, 64
N_CORES = 8
BL = 2          # batches per core
HL = 3          # heads per core
R = BL * S      # 4096 rows per core
KSUB = D // 128  # 6

_CACHE = {}


def _build_nc():
    import concourse.bass as bass  # noqa: F401
    import concourse.tile as tile
    from concourse import bacc, mybir

    f32 = mybir.dt.float32
    bf16 = mybir.dt.bfloat16
    EXP = mybir.ActivationFunctionType.Exp

    nc = bacc.Bacc("TRN2", target_bir_lowering=False, debug=False,
                   num_devices=N_CORES)

    xT_d = nc.dram_tensor("xT", [D, R], bf16, kind="ExternalInput").ap()
    wqk_d = nc.dram_tensor("wqk", [D, 2 * HL * HD], bf16, kind="ExternalInput").ap()
    wv_d = nc.dram_tensor("wv", [D, HL * HD], bf16, kind="ExternalInput").ap()
    wp_d = nc.dram_tensor("wp", [D, D], bf16, kind="ExternalInput").ap()
    bp_d = nc.dram_tensor("bp", [1, D], bf16, kind="ExternalInput").ap()
    mk_d = nc.dram_tensor("mk", [2, 128, 1024], bf16, kind="ExternalInput").ap()
    out_d = nc.dram_tensor("out", [4, 2, 128, D], f32, kind="ExternalOutput").ap()

    RG = [[0, 1, 2, 3, 4, 5, 6, 7]]

    with tile.TileContext(nc) as tc:
        with tc.tile_pool(name="persist", bufs=1) as per, \
             tc.tile_pool(name="dram", bufs=1, space="DRAM") as dram, \
             tc.tile_pool(name="mix_ps", bufs=2, space="PSUM") as mix_ps, \
             tc.tile_pool(name="st_ps", bufs=2, space="PSUM") as st_ps, \
             tc.tile_pool(name="av_ps", bufs=2, space="PSUM") as av_ps, \
             tc.tile_pool(name="pt", bufs=8) as ptp, \
             tc.tile_pool(name="sm", bufs=4) as sm, \
             tc.tile_pool(name="agp", bufs=2) as agp, \
             tc.tile_pool(name="outp", bufs=2) as outp:
            # ---- persistent SBUF tensors -------------------------------
            wqk = per.tile([128, KSUB, 2 * HL * HD], bf16, tag="wqk")
            wv = per.tile([128, KSUB, HL * HD], bf16, tag="wv")
            xT = per.tile([128, KSUB, R], bf16, tag="xT")
            xTr = xT_d.rearrange("(o p) r -> p o r", p=128)
            for j in range(KSUB):  # consumption order for fast PE start
                nc.sync.dma_start(
                    wqk[:, j], wqk_d.rearrange("(o p) c -> p o c", p=128)[:, j])
                nc.sync.dma_start(xT[:, j, 0:512], xTr[:, j, 0:512])
                nc.sync.dma_start(xT[:, j, S:S + 512], xTr[:, j, S:S + 512])
                nc.sync.dma_start(
                    wv[:, j], wv_d.rearrange("(o p) c -> p o c", p=128)[:, j])
            # remaining x chunks, alternating batches (b0rc, b1rc)
            for rc in range(1, 4):
                for b in range(2):
                    r0 = b * S + rc * 512
                    for j in range(KSUB):
                        nc.sync.dma_start(xT[:, j, r0:r0 + 512],
                                          xTr[:, j, r0:r0 + 512])
            masks = per.tile([128, 2, 1024], bf16, tag="mk")
            nc.sync.dma_start(masks[:], mk_d.rearrange("o p c -> p o c"))
            wp = per.tile([128, KSUB, D], bf16, tag="wp")
            nc.sync.dma_start(wp[:], wp_d.rearrange("(o p) c -> p o c", p=128))
            bp_sb = per.tile([1, D], bf16, tag="bp")
            nc.sync.dma_start(bp_sb[:], bp_d[:])
            onesP = per.tile([1, 128], bf16, tag="onesP")
            nc.vector.memset(onesP[:], 1.0)

            # pair p = head p; partitions 0:64 = batch 0, 64:128 = batch 1
            qT = per.tile([128, HL, S], bf16, tag="qT")
            kT = per.tile([128, HL, S], bf16, tag="kT")
            vE = per.tile([128, 2 * 16, HL * 65], bf16, tag="vE")
            nc.vector.memset(vE[:], 1.0)

            # pre-zero the pt pool slots once: diagonal tiles rely on the
            # full-width mask multiply to zero the not-yet-written region,
            # which requires the stale slot contents to be finite.
            pt_init = [ptp.tile([128, 2, 512], bf16, tag="pt", name=f"ptz{i}")
                       for i in range(8)]
            for t in pt_init:
                nc.gpsimd.memset(t[:], 0.0)

            warm_in = dram.tile([8 * 192, 8], bf16, tag="warm_in")
            warm_out = dram.tile([8 * 192, 8], bf16, tag="warm_out")
            a2a_in = [dram.tile([8 * 192, 128], bf16, name=f"a2ai{qb}",
                                tag=f"a2ai{qb}") for qb in range(4)]
            a2a_out = [dram.tile([8 * 192, 128], bf16, name=f"a2ao{qb}",
                                 tag=f"a2ao{qb}") for qb in range(4)]

            # ---- emission helpers --------------------------------------
            def emit_qk_ct(b, rc, ct):
                r0 = b * S + rc * 512
                if True:
                    ps = mix_ps.tile([128, 512], f32, tag="mix", name="ps")
                    for j in range(KSUB):
                        nc.tensor.matmul(
                            ps[:],
                            lhsT=wqk[:, j, ct * 128:(ct + 1) * 128],
                            rhs=xT[:, j, r0:r0 + 512],
                            start=(j == 0), stop=(j == KSUB - 1))
                    for half in range(2):
                        gid = 2 * ct + half
                        dest = qT if gid < 3 else kT
                        pair = gid % 3
                        nc.vector.tensor_copy(
                            dest[b * 64:(b + 1) * 64, pair,
                                 rc * 512:(rc + 1) * 512],
                            ps[half * 64:(half + 1) * 64, :])

            def emit_qk_chunk(b, rc):
                for ct in range(3):
                    emit_qk_ct(b, rc, ct)

            def emit_v_tile(b, rt):
                r0 = b * S + rt * 128
                psv = mix_ps.tile([128, HL * HD], f32, tag="mix", name="psv")
                for j in range(KSUB):
                    nc.tensor.matmul(
                        psv[:], lhsT=xT[:, j, r0:r0 + 128], rhs=wv[:, j, :],
                        start=(j == 0), stop=(j == KSUB - 1))
                dst = vE[:, b * 16 + rt, :].rearrange("p (h c) -> p h c",
                                                      h=HL)
                nc.vector.tensor_copy(
                    dst[:, :, 0:HD],
                    psv[:, :].rearrange("p (h c) -> p h c", c=HD))

            def emit_attn_pair(qb, pair, drain):
                n_k = 4 * (qb + 1)
                n_kp = n_k // 2
                q0 = qb * 512
                avs = [av_ps.tile([65, 512], f32, tag="av", name=f"av{u}")
                       for u in range(2)]
                for kp in range(n_kp):
                    drain(kp)  # due-units for this kp must precede it
                    o = kp - (n_kp - 2)  # diag pair offset; >=0 on diagonal
                    qv0 = 256 if o == 1 else 0  # valid q starts here
                    stps = [st_ps.tile([128, 2, 512], f32, tag="st",
                                       name=f"st{u}") for u in range(2)]
                    for i in range(2):
                        for u in range(2):
                            kt = 2 * kp + i
                            nc.tensor.matmul(
                                stps[u][:, i, qv0:512],
                                lhsT=kT[u * 64:(u + 1) * 64, pair,
                                        kt * 128:(kt + 1) * 128],
                                rhs=qT[u * 64:(u + 1) * 64, pair,
                                       q0 + qv0:q0 + 512],
                                start=True, stop=True)
                    for u in range(2):
                        pt = ptp.tile([128, 2, 512], bf16, tag="pt")
                        nc.scalar.activation(pt[:, :, qv0:512],
                                             stps[u][:, :, qv0:512], EXP,
                                             scale=float(HD) ** -0.5)
                        if o >= 0:
                            # full-width multiply doubles as the zero-fill
                            # of pt[:, :, 0:qv0] (mask is 0 there; the pool
                            # slots are pre-zeroed so stale data is finite)
                            mk2 = masks[:, o, :].rearrange("p (i c) -> p i c",
                                                           i=2)
                            nc.vector.tensor_mul(pt[:], pt[:], mk2[:])
                        for i in range(2):
                            kt = 2 * kp + i
                            nc.tensor.matmul(
                                avs[u][:],
                                lhsT=vE[:, u * 16 + kt,
                                        pair * 65:(pair + 1) * 65],
                                rhs=pt[:, i, :],
                                start=(kp == 0 and i == 0),
                                stop=(kp == n_kp - 1 and i == 1))
                lsbs, recs, bcss, ctxns = [], [], [], []
                for u in range(2):
                    lsbs.append(sm.tile([1, 512], f32, tag="lsb",
                                        name=f"lsb{u}"))
                    nc.vector.tensor_copy(lsbs[u][:], avs[u][64:65, :])
                for u in range(2):
                    recs.append(sm.tile([1, 512], f32, tag="rec",
                                        name=f"rec{u}"))
                    nc.vector.reciprocal_approx_fast(recs[u][:], lsbs[u][:])
                for u in range(2):
                    bcss.append(sm.tile([64, 512], f32, tag="bcs",
                                        name=f"bcs{u}"))
                    nc.gpsimd.partition_broadcast(bcss[u][:], recs[u][:])
                for u in range(2):
                    ctxn = sm.tile([64, 512], bf16, tag="ctxn",
                                   name=f"ctxn{u}")
                    ctxns.append(ctxn)
                    nc.vector.tensor_mul(ctxn[:], avs[u][0:64, :], bcss[u][:])
                    a2v = a2a_in[qb].rearrange("(j f) c -> f j c", f=192)
                    nc.sync.dma_start(
                        a2v[64 * pair:64 * (pair + 1), 4 * u:4 * u + 4, :],
                        ctxns[u].rearrange("p (q c) -> p q c", q=4))

            def emit_a2a(qb):
                nc.gpsimd.collective_compute(
                    "AllToAll", mybir.AluOpType.bypass,
                    ins=[a2a_in[qb][:]], outs=[a2a_out[qb][:]],
                    replica_groups=RG)

            def emit_outproj_blk(qb, blk, ag):
                if blk == 0:
                    nc.sync.dma_start(
                        ag[:], a2a_out[qb].rearrange("(o p) r -> p o r", p=128))
                if True:
                    osb = outp.tile([128, D], f32, tag="osb")
                    for nh in range(2):
                        po = mix_ps.tile([128, 384], f32, tag="mix", name="po")
                        n0 = nh * 384
                        for j in range(KSUB):
                            nc.tensor.matmul(po[:],
                                             lhsT=ag[:, blk * KSUB + j, :],
                                             rhs=wp[:, j, n0:n0 + 384],
                                             start=(j == 0), stop=False)
                        nc.tensor.matmul(po[:], lhsT=onesP[:],
                                         rhs=bp_sb[:, n0:n0 + 384],
                                         start=False, stop=True)
                        nc.vector.tensor_copy(osb[:, n0:n0 + 384], po[:])
                    nc.sync.dma_start(out_d[qb, blk], osb[:])

            # ---- software-pipelined emission ---------------------------
            # warmup collective: absorb ncfw first-call overhead during proj
            nc.sync.dma_start(warm_in[0:128, :], masks[:, 0, 0:8])
            nc.gpsimd.collective_compute(
                "AllToAll", mybir.AluOpType.bypass,
                ins=[warm_in[:]], outs=[warm_out[:]], replica_groups=RG)
            # prologue: everything attention qb0 needs
            emit_qk_chunk(0, 0)
            emit_qk_chunk(1, 0)
            for rt in range(4):
                emit_v_tile(0, rt)
                emit_v_tile(1, rt)

            # filler queue: (deadline=(qb,pair,kp), emit_fn), kept in
            # deadline order; before each k-pair all units due by then are
            # drained (hard ordering requirement: a unit must be emitted
            # before the attention that consumes its output), plus one unit
            # opportunistically per k-pair to spread PE filler.
            from collections import deque
            fq = deque()
            for rc in range(1, 4):
                for b in range(2):
                    for ct in range(3):
                        # qT/kT rows rc needed from (qb=rc, pair0, kp0)
                        fq.append(((rc, 0, 0), lambda b=b, rc=rc, ct=ct:
                                   emit_qk_ct(b, rc, ct)))
                for rt in range(4 * rc, 4 * rc + 4):
                    for b in range(2):
                        # vE row-tile rt consumed at kp=rt//2 of (qb=rc,pair0)
                        dl = (rc, 0, max(0, rt // 2 - 1))
                        fq.append((dl, lambda b=b, rt=rt: emit_v_tile(b, rt)))
            fq = deque(sorted(fq, key=lambda t: t[0]))

            def drain(n, due=None):
                k = 0
                while fq and (k < n or (due and fq[0][0] <= due)):
                    fq.popleft()[1]()
                    k += 1

            for qb in range(4):
                for pair in range(HL):
                    emit_attn_pair(qb, pair, lambda kp, qb=qb, pair=pair:
                                   drain(1, due=(qb, pair, kp)))
                    # out-proj of chunk q consumed ~2 chunks later so the
                    # big late q-blocks keep PE filler (A2A long done)
                    op_sched = {(3, 0): 0, (3, 1): 1}
                    if (qb, pair) in op_sched:
                        q = op_sched[(qb, pair)]
                        ag = agp.tile([128, 2 * KSUB, 128], bf16, tag="ag",
                                      name=f"ag{q}")
                        fq.append(((qb, pair, 98), lambda q=q, a=ag:
                                   emit_outproj_blk(q, 0, a)))
                        fq.append(((qb, pair, 99), lambda q=q, a=ag:
                                   emit_outproj_blk(q, 1, a)))
                if qb == 3:
                    # out-proj of q-block 2 must be EMITTED before the last
                    # AllToAll: instructions placed after a collective on the
                    # same queues end up gated on its completion semaphore.
                    drain(99)
                    ag2 = agp.tile([128, 2 * KSUB, 128], bf16, tag="ag",
                                   name="ag2")
                    emit_outproj_blk(2, 0, ag2)
                    emit_outproj_blk(2, 1, ag2)
                emit_a2a(qb)
            ag3 = agp.tile([128, 2 * KSUB, 128], bf16, tag="ag", name="ag3")
            emit_outproj_blk(3, 0, ag3)
            emit_outproj_blk(3, 1, ag3)

    nc.compile()
    return nc


def _get_nc():
    if "nc" not in _CACHE:
        _CACHE["nc"] = _build_nc()
    return _CACHE["nc"]


def _masks_np():
    k = np.arange(128)[:, None]
    q = np.arange(512)[None, :]
    tiles = [(q >= k + 128 * t) for t in range(4)]
    m = np.stack([np.concatenate([tiles[2 * o], tiles[2 * o + 1]], axis=1)
                  for o in range(2)])
    return m.astype(BF16)


def _prep_in_maps(x, Wq, Wk, Wv, Wp, bp):
    x = np.asarray(x, dtype=np.float32)
    mk = _masks_np()
    wp_full = np.asarray(Wp).astype(BF16)
    bp_row = np.asarray(bp, dtype=np.float32).reshape(1, D).astype(BF16)
    xT_bg = []
    for bg in range(2):
        xl = x[2 * bg:2 * bg + 2].reshape(R, D)
        xT_bg.append(np.ascontiguousarray(xl.T).astype(BF16))
    wqk_hg, wv_hg = [], []
    for hg in range(4):
        hs = slice(192 * hg, 192 * (hg + 1))
        wqk_hg.append(np.concatenate(
            [np.asarray(Wq)[:, hs], np.asarray(Wk)[:, hs]], axis=1).astype(BF16))
        wv_hg.append(np.asarray(Wv)[:, hs].astype(BF16))
    in_maps = []
    for c in range(N_CORES):
        bg, hg = c // 4, c % 4
        in_maps.append({
            "xT": xT_bg[bg],
            "wqk": wqk_hg[hg],
            "wv": wv_hg[hg],
            "wp": wp_full,
            "bp": bp_row,
            "mk": mk,
        })
    return in_maps


def kernel(x, Wq, Wk, Wv, Wp, bp):
    from concourse import bass_utils

    nc = _get_nc()
    in_maps = _prep_in_maps(x, Wq, Wk, Wv, Wp, bp)
    res = bass_utils.run_bass_kernel_spmd(nc, in_maps,
                                          core_ids=list(range(N_CORES)))
    out = np.empty((B, S, D), np.float32)
    for c in range(N_CORES):
        sh = res.results[c]["out"]  # [4 chunks, 2 blocks, 128, D]
        for qb in range(4):
            for blk in range(2):
                batch = 2 * blk + c // 4
                s0 = 512 * qb + 128 * (c % 4)
                out[batch, s0:s0 + 128] = sh[qb, blk]
    return out

